# revision 7
# baseline (speedup 1.0000x reference)
"""BatchAllTripletLoss kernel for 8 Trainium2 NeuronCores.

Reference computation:
    pd = pairwise_euclidean(rep)                        # [512, 512]
    tl[a,p,k] = relu(pd[a,p] - pd[a,k] + 5.0) * mask    # [512, 512, 512]
    loss = sum(tl) / (count(tl > eps) + eps)

The mask (p!=a, k!=a, p!=k, label[p]==label[a], label[k]!=label[a])
collapses: label[p]==label[a] and label[k]!=label[a] imply p!=k and k!=a,
so valid triplets are exactly (anchor-positive pairs) x (k with a
different label).  With 64 labels over 512 rows there are only ~4100
(a,p) pairs, so instead of a dense [N,N,N] sweep each core processes its
anchors' pairs as rows of [128-pair, 512-k] tiles:

  per core (64 anchors):
    d[64,512]   = sqrt(relu(aug-matmul))            PE + DVE + ACT
    ym          = d + BIG*same_label                DVE
    per pair-tile t:
      G         = sel_t.T @ [ym | d]                PE one-hot row gather
      x[p]      = sum_k (iota==pidx)*Gd             DVE (bias gather)
      S_t[p]    = sum_k relu(x + margin - Gym)      ACT accum
      C_t[p]    = sum_k (Gym < x + margin)          DVE accum
    out[1,2*Tp] = ones.T @ [S | C]                  PE partition sum

Anchors are block-sharded 64 per core; the 8 partial (sum, count) pairs
are reduced on the host (the all-reduce step of the sharding hint).
Host-side prep is integer/mask logic only (pair enumeration, one-hot
selectors, label masks); all float math runs on device.
"""

import numpy as np

import concourse.bass as bass
import concourse.tile as tile
from concourse import bacc, mybir
from concourse.bass_utils import run_bass_kernel_spmd
from concourse.masks import make_identity

F32 = mybir.dt.float32
AF = mybir.ActivationFunctionType
OP = mybir.AluOpType

N = 512          # rows
D = 256          # embedding dim
NCORES = 8
A = N // NCORES  # anchors per core
MARGIN = 5.0
EPS = 1e-16
BIG = 1e30

_cache = {}


def _build(Tp: int):
    """Build the (uniform, SPMD) per-core Bass program for Tp pair tiles."""
    nc = bacc.Bacc(None, target_bir_lowering=False)

    rep_d = nc.declare_dram_parameter("rep", [N, D], F32, isOutput=False)
    repa_d = nc.declare_dram_parameter("repa", [A, D], F32, isOutput=False)
    bigneg_d = nc.declare_dram_parameter("bigneg", [A, N], F32, isOutput=False)
    sel_d = nc.declare_dram_parameter("sel", [A, Tp * 128], F32, isOutput=False)
    pm_d = nc.declare_dram_parameter("pm", [128, 2 * Tp], F32, isOutput=False)
    out_d = nc.declare_dram_parameter("out", [1, 2 * Tp], F32, isOutput=True)

    with tile.TileContext(nc) as tc:
        with (
            tc.tile_pool(name="singles", bufs=1) as sg,
            tc.tile_pool(name="scr", bufs=2) as scr,
            tc.tile_pool(name="xs", bufs=3) as xs,
            tc.tile_pool(name="ppt", bufs=2, space="PSUM") as ppt,
            tc.tile_pool(name="ppa", bufs=1, space="PSUM") as ppa,
            tc.tile_pool(name="ppg", bufs=2, space="PSUM") as ppg,
            tc.tile_pool(name="ppd", bufs=1, space="PSUM") as ppd,
        ):
            ident = sg.tile([128, 128], F32)
            make_identity(nc, ident[:])
            iota_f = sg.tile([128, N], F32)
            nc.gpsimd.iota(
                iota_f[:], [[1, N]], channel_multiplier=0,
                allow_small_or_imprecise_dtypes=True,
            )
            ones = sg.tile([128, 1], F32)
            nc.vector.memset(ones[:], 1.0)
            onesrow = sg.tile([1, A], F32)
            nc.vector.memset(onesrow[:], 1.0)

            # input loads
            rep_s = sg.tile([128, 4, D], F32)      # rep[t*128+p, :] -> rep_s[p, t, :]
            for t in range(4):
                nc.sync.dma_start(rep_s[:, t, :], rep_d[t * 128:(t + 1) * 128, :])
            repa_s = sg.tile([A, D], F32)
            nc.sync.dma_start(repa_s[:], repa_d[:])
            bigneg_s = sg.tile([A, N], F32)
            nc.sync.dma_start(bigneg_s[:], bigneg_d[:])
            sel_s = sg.tile([A, Tp * 128], F32)
            nc.sync.dma_start(sel_s[:], sel_d[:])
            pm_s = sg.tile([128, 2 * Tp], F32)     # [:, :Tp] pidx, [:, Tp:] margin
            nc.sync.dma_start(pm_s[:], pm_d[:])

            # transpose rep -> repT chunks [128, 512] (c = d-chunk)
            repT = []
            for c in range(2):
                rt = sg.tile([128, N], F32, tag=f"repT{c}")
                repT.append(rt)
                for t in range(4):
                    pt = ppt.tile([128, 128], F32, tag="tr")
                    nc.tensor.transpose(
                        pt[:], rep_s[:, t, c * 128:(c + 1) * 128], ident[:]
                    )
                    nc.vector.tensor_copy(rt[:, t * 128:(t + 1) * 128], pt[:])

            # sq_row[1, j] = ||rep_j||^2 via ones.T @ (repT * repT)
            sqrow_p = ppa.tile([1, N], F32, tag="aux")
            for c in range(2):
                sqc = scr.tile([128, N], F32, tag="sqx")
                nc.vector.tensor_mul(sqc[:], repT[c][:], repT[c][:])
                nc.tensor.matmul(
                    sqrow_p[:], ones[:], sqc[:], start=(c == 0), stop=(c == 1)
                )
            sqrow = sg.tile([1, N], F32)
            nc.vector.tensor_copy(sqrow[:], sqrow_p[:])

            # sq_anch[64,1] = ||rep_a||^2
            # (tensor_tensor_reduce crashes the exec unit on this toolchain;
            # scalar_tensor_tensor with add-accumulate is the safe spelling)
            sqa_scr = scr.tile([A, D], F32, tag="sqa")
            sqanch = sg.tile([A, 1], F32)
            nc.vector.scalar_tensor_tensor(
                out=sqa_scr[:], in0=repa_s[:], scalar=1.0, in1=repa_s[:],
                op0=OP.mult, op1=OP.mult, accum_out=sqanch[:],
            )

            # repTa chunks scaled by -2: [128, 64] each
            negTa = []
            for c in range(2):
                pta = ppt.tile([128, A], F32, tag="tr")
                nc.tensor.transpose(
                    pta[:], repa_s[:, c * 128:(c + 1) * 128], ident[0:A, 0:A]
                )
                nt = sg.tile([128, A], F32, tag=f"negTa{c}")
                negTa.append(nt)
                nc.vector.tensor_scalar_mul(nt[:], pta[:], -2.0)

            # d2[a, j] = sq_a + sq_j - 2*dot  (aug matmul + bias add)
            d2_p = ppd.tile([A, N], F32, tag="d2")
            nc.tensor.matmul(d2_p[:], negTa[0][:], repT[0][:], start=True, stop=False)
            nc.tensor.matmul(d2_p[:], negTa[1][:], repT[1][:], start=False, stop=False)
            nc.tensor.matmul(d2_p[:], onesrow[:], sqrow[:], start=False, stop=True)

            d2c = sg.tile([A, N], F32)
            nc.vector.tensor_scalar(
                d2c[:], d2_p[:], sqanch[:], 0.0, OP.add, OP.max
            )

            # ymd = [ym | d]: d = sqrt(d2c); ym = bigpos + d
            ymd = sg.tile([A, 2 * N], F32)
            nc.scalar.activation(ymd[:, N:2 * N], d2c[:], AF.Sqrt)
            nc.vector.tensor_add(ymd[:, 0:N], bigneg_s[:], ymd[:, N:2 * N])

            # pair tiles
            SC = sg.tile([128, 2 * Tp], F32)
            for t in range(Tp):
                selt = sel_s[:, t * 128:(t + 1) * 128]
                gy = ppg.tile([128, N], F32, tag="gy")
                nc.tensor.matmul(gy[:], selt, ymd[:, 0:N], start=True, stop=True)
                gd = ppg.tile([128, N], F32, tag="gd")
                nc.tensor.matmul(gd[:], selt, ymd[:, N:2 * N], start=True, stop=True)

                stt = scr.tile([128, N], F32, tag="stt")
                xv = xs.tile([128, 1], F32, tag="xv")
                nc.vector.scalar_tensor_tensor(
                    out=stt[:], in0=iota_f[:], scalar=pm_s[:, t:t + 1], in1=gd[:],
                    op0=OP.is_equal, op1=OP.mult, accum_out=xv[:],
                )
                xp = xs.tile([128, 1], F32, tag="xp")
                nc.vector.tensor_scalar(
                    xp[:], xv[:], pm_s[:, Tp + t:Tp + t + 1], None, OP.add
                )

                rel = scr.tile([128, N], F32, tag="rel")
                nc.scalar.activation(
                    rel[:], gy[:], AF.Relu, bias=xp[:], scale=-1.0,
                    accum_out=SC[:, t:t + 1],
                )
                cnt = scr.tile([128, N], F32, tag="cnt")
                nc.vector.tensor_scalar(
                    cnt[:], gy[:], xp[:], 0.0, OP.is_lt, OP.add,
                    accum_out=SC[:, Tp + t:Tp + t + 1],
                )

            # partition-sum S and C columns -> [1, 2*Tp]
            # (reuses the transpose pool's tag: its slots are long dead)
            fin_p = ppt.tile([1, 2 * Tp], F32, tag="tr")
            nc.tensor.matmul(fin_p[:], ones[:], SC[:], start=True, stop=True)
            outsb = sg.tile([1, 2 * Tp], F32)
            nc.vector.tensor_copy(outsb[:], fin_p[:])
            nc.sync.dma_start(out_d[:], outsb[:])

    nc.finalize()
    return nc


def _prep(rep: np.ndarray, labels: np.ndarray):
    """Host-side integer/mask prep: shard anchors, enumerate (a,p) pairs."""
    rep = np.ascontiguousarray(np.asarray(rep, dtype=np.float32))
    labels = np.asarray(labels)
    same = labels[:, None] == labels[None, :]

    pairs = []
    for c in range(NCORES):
        base = c * A
        prs = [
            (j, p)
            for j in range(A)
            for p in np.nonzero(same[base + j])[0]
            if p != base + j
        ]
        pairs.append(prs)
    Tp = max(1, max((len(p) + 127) // 128 for p in pairs))

    in_maps = []
    for c in range(NCORES):
        base = c * A
        bigneg = np.where(same[base:base + A], BIG, 0.0).astype(np.float32)
        sel = np.zeros((A, Tp * 128), np.float32)
        pm = np.zeros((128, 2 * Tp), np.float32)
        pm[:, Tp:] = -BIG
        for i, (j, p) in enumerate(pairs[c]):
            t, r = divmod(i, 128)
            sel[j, i] = 1.0
            pm[r, t] = p
            pm[r, Tp + t] = MARGIN
        in_maps.append({
            "rep": rep,
            "repa": rep[base:base + A],
            "bigneg": bigneg,
            "sel": sel,
            "pm": pm,
        })
    return Tp, in_maps


def _run(rep, labels, trace=False):
    Tp, in_maps = _prep(rep, labels)
    if Tp not in _cache:
        _cache[Tp] = _build(Tp)
    nc = _cache[Tp]
    res = run_bass_kernel_spmd(nc, in_maps, list(range(NCORES)), trace=trace)
    outs = np.stack([res.results[c]["out"][0] for c in range(NCORES)])  # [8, 2*Tp]
    S = float(outs[:, :Tp].sum())
    C = float(outs[:, Tp:].sum())
    loss = np.float32(S / (C + EPS))
    return np.asarray(loss, dtype=np.float32), res


def kernel(rep, labels):
    loss, _ = _run(rep, labels, trace=False)
    return loss


# revision 13
# speedup vs baseline: 1.1361x; 1.1361x over previous
"""BatchAllTripletLoss kernel for 8 Trainium2 NeuronCores.

Reference computation:
    pd = pairwise_euclidean(rep)                        # [512, 512]
    tl[a,p,k] = relu(pd[a,p] - pd[a,k] + 5.0) * mask    # [512, 512, 512]
    loss = sum(tl) / (count(tl > eps) + eps)

The mask (p!=a, k!=a, p!=k, label[p]==label[a], label[k]!=label[a])
collapses: label[p]==label[a] and label[k]!=label[a] imply p!=k and k!=a,
so valid triplets are exactly (anchor-positive pairs) x (k with a
different label).  With 64 labels over 512 rows there are only ~4100
(a,p) pairs, so instead of a dense [N,N,N] sweep each core processes its
anchors' pairs as rows of [128-pair, 512-k] tiles:

  per core (64 anchors):
    d[64,512]   = sqrt(relu(aug-matmul))            PE + DVE + ACT
    ym          = d + BIGM*same_label               DVE
    per pair-tile t:
      Gym       = sel_t.T @ ym                      PE one-hot row gather
      x[p]      = sum_k (iota==pidx)*Gym            DVE; = d[a,p] + BIGM
      xp        = x + (margin - BIGM)               DVE
      S_t[p]    = sum_k relu(xp - Gym)              ACT accum
      C_t[p]    = sum_k (Gym < xp)                  DVE accum
    out[1,2*Tp] = ones.T @ [S | C]                  PE partition sum

BIGM = 4096 both masks out same-label k columns (xp <= ~50 << 4096 so
relu/count give exactly 0) and carries the bias through the gather: the
quantization of d stored as d+4096 is 2^-11, noise ~1e-4 relative on the
final sums.  Anchors are block-sharded 64 per core; the 8 partial
(sum, count) pairs are reduced on the host (the all-reduce of the
sharding hint).  Host-side prep is integer/mask logic only (pair
enumeration, one-hot selectors, label masks); all float math runs on
device.
"""

import numpy as np

import concourse.bass as bass
import concourse.tile as tile
from concourse import bacc, mybir
from concourse.bass_utils import run_bass_kernel_spmd
from concourse.masks import make_identity

F32 = mybir.dt.float32
AF = mybir.ActivationFunctionType
OP = mybir.AluOpType

N = 512          # rows
D = 256          # embedding dim
NCORES = 8
A = N // NCORES  # anchors per core
MARGIN = 5.0
EPS = 1e-16
BIG = 1e30       # pad-pair kill value
BIGM = 4096.0    # same-label mask / bias carrier (power of two)

_cache = {}


def _build(Tp: int):
    """Build the (uniform, SPMD) per-core Bass program for Tp pair tiles."""
    nc = bacc.Bacc(None, target_bir_lowering=False)

    rep_d = nc.declare_dram_parameter("rep", [N, D], F32, isOutput=False)
    repa_d = nc.declare_dram_parameter("repa", [A, D], F32, isOutput=False)
    bigm_d = nc.declare_dram_parameter("bigm", [A, N], F32, isOutput=False)
    sel_d = nc.declare_dram_parameter("sel", [A, Tp * 128], F32, isOutput=False)
    pm_d = nc.declare_dram_parameter("pm", [128, 2 * Tp], F32, isOutput=False)
    out_d = nc.declare_dram_parameter("out", [1, 2 * Tp], F32, isOutput=True)

    with tile.TileContext(nc) as tc:
        with (
            tc.tile_pool(name="singles", bufs=1) as sg,
            tc.tile_pool(name="scr", bufs=2) as scr,
            tc.tile_pool(name="xs", bufs=3) as xs,
            tc.tile_pool(name="ppt", bufs=2, space="PSUM") as ppt,
            tc.tile_pool(name="ppa", bufs=1, space="PSUM") as ppa,
            tc.tile_pool(name="ppg", bufs=3, space="PSUM") as ppg,
            tc.tile_pool(name="ppd", bufs=1, space="PSUM") as ppd,
        ):
            ident = sg.tile([128, 128], F32)
            make_identity(nc, ident[:])
            iota_f = sg.tile([128, N], F32)
            nc.gpsimd.iota(
                iota_f[:], [[1, N]], channel_multiplier=0,
                allow_small_or_imprecise_dtypes=True,
            )
            ones = sg.tile([128, 1], F32)
            nc.vector.memset(ones[:], 1.0)
            onesrow = sg.tile([1, A], F32)
            nc.vector.memset(onesrow[:], 1.0)

            # input loads, spread across HWDGE queues via issuing engine
            rep_s = sg.tile([128, 4, D], F32)      # rep[t*128+p, :] -> rep_s[p, t, :]
            dma_eng = [nc.sync, nc.scalar, nc.sync, nc.scalar]
            for t in range(4):
                dma_eng[t].dma_start(rep_s[:, t, :], rep_d[t * 128:(t + 1) * 128, :])
            repa_s = sg.tile([A, D], F32)
            nc.gpsimd.dma_start(repa_s[:], repa_d[:])
            bigm_s = sg.tile([A, N], F32)
            nc.gpsimd.dma_start(bigm_s[:], bigm_d[:])
            sel_s = sg.tile([A, Tp * 128], F32)
            nc.sync.dma_start(sel_s[:], sel_d[:])
            pm_s = sg.tile([128, 2 * Tp], F32)     # [:, :Tp] pidx, [:, Tp:] margin
            nc.scalar.dma_start(pm_s[:], pm_d[:])

            # sq4[p, t] = ||rep_{t*128+p}||^2, transposed to [4, 128] rows for
            # the K=1 aug matmuls below
            sq4 = sg.tile([128, 4], F32)
            for t in range(4):
                s4s = scr.tile([128, D], F32, tag="s4s")
                nc.vector.scalar_tensor_tensor(
                    out=s4s[:], in0=rep_s[:, t, :], scalar=1.0, in1=rep_s[:, t, :],
                    op0=OP.mult, op1=OP.mult, accum_out=sq4[:, t:t + 1],
                )
            sqrow_p = ppa.tile([1, N], F32, tag="aux")
            for t in range(4):
                nc.tensor.transpose(
                    sqrow_p[0:1, t * 128:(t + 1) * 128], sq4[:, t:t + 1], ident[:]
                )
            sqrow = sg.tile([1, N], F32)
            nc.vector.tensor_copy(sqrow[:], sqrow_p[:])

            # sq_anch[64,1] = ||rep_a||^2
            sqa_scr = scr.tile([A, D], F32, tag="sqa")
            sqanch = sg.tile([A, 1], F32)
            nc.vector.scalar_tensor_tensor(
                out=sqa_scr[:], in0=repa_s[:], scalar=1.0, in1=repa_s[:],
                op0=OP.mult, op1=OP.mult, accum_out=sqanch[:],
            )

            # transpose rep -> repT chunks [128, 512] (c = d-chunk)
            repT = []
            for c in range(2):
                rt = sg.tile([128, N], F32, tag=f"repT{c}")
                repT.append(rt)
                for t in range(4):
                    pt = ppt.tile([128, 128], F32, tag="tr")
                    nc.tensor.transpose(
                        pt[:], rep_s[:, t, c * 128:(c + 1) * 128], ident[:]
                    )
                    nc.vector.tensor_copy(rt[:, t * 128:(t + 1) * 128], pt[:])

            # repTa chunks scaled by -2: [128, 64] each
            negTa = []
            for c in range(2):
                pta = ppt.tile([128, A], F32, tag="tr")
                nc.tensor.transpose(
                    pta[:], repa_s[:, c * 128:(c + 1) * 128], ident[0:A, 0:A]
                )
                nt = sg.tile([128, A], F32, tag=f"negTa{c}")
                negTa.append(nt)
                nc.vector.tensor_scalar_mul(nt[:], pta[:], -2.0)

            # d2[a, j] = sq_a + sq_j - 2*dot  (aug matmul + bias add)
            d2_p = ppd.tile([A, N], F32, tag="d2")
            nc.tensor.matmul(d2_p[:], negTa[0][:], repT[0][:], start=True, stop=False)
            nc.tensor.matmul(d2_p[:], negTa[1][:], repT[1][:], start=False, stop=False)
            nc.tensor.matmul(d2_p[:], onesrow[:], sqrow[:], start=False, stop=True)

            d2c = sg.tile([A, N], F32)
            nc.vector.tensor_scalar(
                d2c[:], d2_p[:], sqanch[:], 0.0, OP.add, OP.max
            )

            # ym = sqrt(d2c) + BIGM*same
            dtmp = scr.tile([A, N], F32, tag="dtmp")
            nc.scalar.activation(dtmp[:], d2c[:], AF.Sqrt)
            ym = sg.tile([A, N], F32)
            nc.vector.tensor_add(ym[:], bigm_s[:], dtmp[:])

            # pair tiles
            SC = sg.tile([128, 2 * Tp], F32)
            for t in range(Tp):
                selt = sel_s[:, t * 128:(t + 1) * 128]
                gy = ppg.tile([128, N], F32, tag="gy")
                nc.tensor.matmul(gy[:], selt, ym[:], start=True, stop=True)

                stt = scr.tile([128, N], F32, tag="stt")
                xv = xs.tile([128, 1], F32, tag="xv")
                nc.vector.scalar_tensor_tensor(
                    out=stt[:], in0=iota_f[:], scalar=pm_s[:, t:t + 1], in1=gy[:],
                    op0=OP.is_equal, op1=OP.mult, accum_out=xv[:],
                )
                xp = xs.tile([128, 1], F32, tag="xp")
                nc.vector.tensor_scalar(
                    xp[:], xv[:], pm_s[:, Tp + t:Tp + t + 1], None, OP.add
                )

                rel = scr.tile([128, N], F32, tag="rel")
                nc.scalar.activation(
                    rel[:], gy[:], AF.Relu, bias=xp[:], scale=-1.0,
                    accum_out=SC[:, t:t + 1],
                )
                cnt = scr.tile([128, N], F32, tag="cnt")
                nc.vector.tensor_scalar(
                    cnt[:], gy[:], xp[:], 0.0, OP.is_lt, OP.add,
                    accum_out=SC[:, Tp + t:Tp + t + 1],
                )

            # partition-sum S and C columns -> [1, 2*Tp]
            # (reuses the transpose pool's tag: its slots are long dead)
            fin_p = ppt.tile([1, 2 * Tp], F32, tag="tr")
            nc.tensor.matmul(fin_p[:], ones[:], SC[:], start=True, stop=True)
            outsb = sg.tile([1, 2 * Tp], F32)
            nc.vector.tensor_copy(outsb[:], fin_p[:])
            nc.sync.dma_start(out_d[:], outsb[:])

    nc.finalize()
    return nc


def _prep(rep: np.ndarray, labels: np.ndarray):
    """Host-side integer/mask prep: shard anchors, enumerate (a,p) pairs."""
    rep = np.ascontiguousarray(np.asarray(rep, dtype=np.float32))
    labels = np.asarray(labels)
    same = labels[:, None] == labels[None, :]

    pairs = []
    for c in range(NCORES):
        base = c * A
        prs = [
            (j, p)
            for j in range(A)
            for p in np.nonzero(same[base + j])[0]
            if p != base + j
        ]
        pairs.append(prs)
    Tp = max(1, max((len(p) + 127) // 128 for p in pairs))

    in_maps = []
    for c in range(NCORES):
        base = c * A
        bigm = np.where(same[base:base + A], BIGM, 0.0).astype(np.float32)
        sel = np.zeros((A, Tp * 128), np.float32)
        pm = np.zeros((128, 2 * Tp), np.float32)
        pm[:, Tp:] = -BIG
        for i, (j, p) in enumerate(pairs[c]):
            t, r = divmod(i, 128)
            sel[j, i] = 1.0
            pm[r, t] = p
            pm[r, Tp + t] = MARGIN - BIGM
        in_maps.append({
            "rep": rep,
            "repa": rep[base:base + A],
            "bigm": bigm,
            "sel": sel,
            "pm": pm,
        })
    return Tp, in_maps


def _run(rep, labels, trace=False):
    Tp, in_maps = _prep(rep, labels)
    if Tp not in _cache:
        _cache[Tp] = _build(Tp)
    nc = _cache[Tp]
    res = run_bass_kernel_spmd(nc, in_maps, list(range(NCORES)), trace=trace)
    outs = np.stack([res.results[c]["out"][0] for c in range(NCORES)])  # [8, 2*Tp]
    S = float(outs[:, :Tp].sum())
    C = float(outs[:, Tp:].sum())
    loss = np.float32(S / (C + EPS))
    return np.asarray(loss, dtype=np.float32), res


def kernel(rep, labels):
    loss, _ = _run(rep, labels, trace=False)
    return loss
